# revision 7
# baseline (speedup 1.0000x reference)
"""Embedding-lookup kernel for 8 TRN2 NeuronCores.

Computes out[b, :] = z[b, :] + a[:, idx[b]] * scale[b] for B=1M rows.

Strategy (data-parallel over batch):
  - Each of the 8 cores handles BC = B/8 = 131072 rows.
  - The (128, 512) table `a` lives in SBUF (2KB/partition); the gather
    happens ON-CHIP: GPSIMD's ap_gather reads G[d, i] = a[d, idx[i]]
    (output free dim = batch, partitions = z_dim), TensorE transposes
    each 128x128 tile into PSUM ([batch, z_dim] layout), and DVE fuses
    (g * scale + z) via scalar_tensor_tensor straight out of PSUM.
  - z / out stream contiguously (32KB per-partition runs); the only
    DMA traffic is z in + out back (+2.5MB idx/scale per core), so the
    kernel is limited by the ~128MB/core stream instead of the 131072
    random 512-byte descriptor-bound table reads the previous
    dma_gather version paid (~1.2ms -> this version ~0.5ms).
  - Indices are pre-permuted on the host so ap_gather position
    i = tt*128 + p holds batch row p*t + tt; after the tile transpose,
    PSUM partition p, tile tt lines up with z's SBUF layout.

Raw Bass (no Tile framework), manually triple-buffered: SP issues HWDGE
loads/stores, GPSIMD the gathers, PE the transposes, DVE the fused FMA.
Semaphores count monotonically; NRT's sema_reset preamble re-zeroes
them before every execution.
"""

import contextlib

import numpy as np

import concourse.bass as bass
import concourse.mybir as mybir
from concourse import library_config
from concourse.bass_utils import run_bass_kernel_spmd

F32 = mybir.dt.float32
I16 = mybir.dt.int16

B = 1048576
Z = 128
K = 512
NCORES = 8
BC = B // NCORES  # rows per core
NBUF = 3


def build_program(bc=BC, chunk=8192, repeats=1, gather_n=8192, _ablate=(),
                  nbuf=None, npsum=8, wgrp=4, bench_io=False):
    """Build the single-core Bass program (same module runs SPMD on all cores).

    repeats > 1 re-runs the whole computation (statically unrolled) for
    benchmarking: wall-time slope over repeats isolates on-device time.
    gather_n: indices per ap_gather instruction.
    wgrp: transpose/fma tiles per semaphore wait (amortizes seq waits).
    bench_io: timing-only variant - z/out become Internal DRAM scratch
    (same instruction stream, garbage data) so per-execution transfers
    shrink from ~1.5GB to ~20MB; a tiny `done` external output provides
    completion. Numerically meaningless, structurally identical.
    """
    t = chunk // 128  # column blocks per chunk
    nch = bc // chunk  # chunks per core
    assert bc % chunk == 0 and chunk % 128 == 0
    NBUF = nbuf or globals()["NBUF"]
    gather_n = min(gather_n, chunk)
    nsub = chunk // gather_n
    assert chunk % gather_n == 0 and gather_n % 128 == 0
    total = nch * repeats
    ntiles = total * t  # global 128-row tile count

    nc = bass.Bass()
    a_p = nc.declare_dram_parameter("a", [Z, K], F32, isOutput=False)
    ident_p = nc.declare_dram_parameter("identw", [128, 128], F32, isOutput=False)
    if bench_io:
        z = nc.dram_tensor("z", [bc, Z], F32, kind="Internal")
        idxw = nc.declare_dram_parameter("idxw", [nch, 128, chunk // 16], I16, isOutput=False)
        scw = nc.declare_dram_parameter("scw", [nch, 128, t], F32, isOutput=False)
        out = nc.dram_tensor("out", [bc, Z], F32, kind="Internal")
        done = nc.declare_dram_parameter("done", [1, 64], I16, isOutput=True)
    else:
        z = nc.declare_dram_parameter("z", [bc, Z], F32, isOutput=False)
        idxw = nc.declare_dram_parameter("idxw", [nch, 128, chunk // 16], I16, isOutput=False)
        scw = nc.declare_dram_parameter("scw", [nch, 128, t], F32, isOutput=False)
        out = nc.declare_dram_parameter("out", [bc, Z], F32, isOutput=True)
        done = None

    # chunk-row b = p*t + tt lives at SBUF (partition p, column block tt)
    z_v = z.ap().rearrange("(c p tt) d -> c p (tt d)", p=128, tt=t)
    o_v = out.ap().rearrange("(c p tt) d -> c p (tt d)", p=128, tt=t)

    with contextlib.ExitStack() as ctx:
        zts = [
            ctx.enter_context(nc.sbuf_tensor(f"zt{i}", [128, t * Z], F32))
            for i in range(NBUF)
        ]
        gts = [
            ctx.enter_context(nc.sbuf_tensor(f"gt{i}", [128, chunk], F32))
            for i in range(NBUF)
        ]
        idxts = [
            ctx.enter_context(nc.sbuf_tensor(f"idxt{i}", [128, chunk // 16], I16))
            for i in range(NBUF)
        ]
        scts = [
            ctx.enter_context(nc.sbuf_tensor(f"sct{i}", [128, t], F32))
            for i in range(NBUF)
        ]
        tab = ctx.enter_context(nc.sbuf_tensor("tab", [128, K], F32))
        ident = ctx.enter_context(nc.sbuf_tensor("ident", [128, 128], F32))
        pts = [
            ctx.enter_context(nc.psum_tensor(f"pt{i}", [128, 128], F32))
            for i in range(npsum)
        ]
        sem_is = [ctx.enter_context(nc.semaphore(f"sem_is{i}")) for i in range(NBUF)]
        sem_z = [ctx.enter_context(nc.semaphore(f"sem_z{i}")) for i in range(NBUF)]
        sem_g = [ctx.enter_context(nc.semaphore(f"sem_g{i}")) for i in range(NBUF)]
        sem_o = [ctx.enter_context(nc.semaphore(f"sem_o{i}")) for i in range(NBUF)]
        sem_tab = ctx.enter_context(nc.semaphore("sem_tab"))
        sem_id = ctx.enter_context(nc.semaphore("sem_id"))
        sem_t = ctx.enter_context(nc.semaphore("sem_t"))  # PE transposes, +1/tile
        sem_f = ctx.enter_context(nc.semaphore("sem_f"))  # DVE fmas, +1/tile
        block = ctx.enter_context(nc.Block())

        def nuses(j):  # completed uses of slot j%NBUF's sems after chunk j
            return j // NBUF + 1

        @block.sync
        def _(sync):
            sync.dma_start(out=tab[:], in_=a_p.ap()).then_inc(sem_tab, 16)
            sync.dma_start(out=ident[:], in_=ident_p.ap()).then_inc(sem_id, 16)
            for k in range(total):
                c = k % nch
                b = k % NBUF
                if k >= NBUF:
                    # slot reuse: gather(k-NBUF), fma(k-NBUF), out(k-NBUF) done
                    sync.wait_ge(sem_g[b], nsub * nuses(k - NBUF))
                    sync.wait_ge(sem_f, (k - NBUF + 1) * t)
                    sync.wait_ge(sem_o[b], 16 * nuses(k - NBUF))
                sync.dma_start(out=idxts[b][:], in_=idxw.ap()[c]).then_inc(sem_is[b], 16)
                sync.dma_start(out=scts[b][:], in_=scw.ap()[c]).then_inc(sem_is[b], 16)
                sync.dma_start(out=zts[b][:], in_=z_v[c]).then_inc(sem_z[b], 16)
                if k >= 2:
                    j = k - 2  # store lags loads by 2 chunks
                    sync.wait_ge(sem_f, (j + 1) * t)
                    sync.dma_start(out=o_v[j % nch], in_=zts[j % NBUF][:]).then_inc(
                        sem_o[j % NBUF], 16
                    )
            for j in range(max(total - 2, 0), total):
                sync.wait_ge(sem_f, (j + 1) * t)
                sync.dma_start(out=o_v[j % nch], in_=zts[j % NBUF][:]).then_inc(
                    sem_o[j % NBUF], 16
                )
            for b in range(NBUF):
                count_b = len([j for j in range(total) if j % NBUF == b])
                if count_b:
                    sync.wait_ge(sem_o[b], 16 * count_b)
            if done is not None:
                sync.dma_start(out=done.ap(), in_=idxts[0][:1, :64]).then_inc(
                    sem_is[0], 16
                )
                sync.wait_ge(sem_is[0], 32 * nuses(total - 1) + 16)

        @block.gpsimd
        def _(gpsimd):
            gpsimd.load_library(library_config.ap_gather)
            gpsimd.wait_ge(sem_tab, 16)
            for k in range(total):
                b = k % NBUF
                gpsimd.wait_ge(sem_is[b], 32 * nuses(k))
                if k >= NBUF:
                    # gt slot reuse: PE finished transposing chunk k-NBUF
                    gpsimd.wait_ge(sem_t, (k - NBUF + 1) * t)
                for s in range(nsub):
                    gpsimd.ap_gather(
                        out_ap=gts[b][:, s * gather_n : (s + 1) * gather_n],
                        in_ap=tab[:],
                        idxs_ap=idxts[b][:, s * (gather_n // 16) : (s + 1) * (gather_n // 16)],
                        channels=128,
                        num_elems=K,
                        d=1,
                        num_idxs=gather_n,
                    ).then_inc(sem_g[b], 1)

        @block.tensor
        def _(tensor):
            tensor.wait_ge(sem_id, 16)
            for k in range(total):
                b = k % NBUF
                spt = gather_n // 128  # tiles covered per sub-gather
                for tt in range(t):
                    g = k * t + tt  # global tile index
                    if tt % wgrp == 0:
                        hi = min(tt + wgrp, t)
                        # gathers covering tiles tt..hi-1 done
                        need = (hi + spt - 1) // spt
                        tensor.wait_ge(sem_g[b], nsub * (nuses(k) - 1) + need)
                        # PSUM tile reuse: fma consumed tile g+wgrp-1-npsum
                        lim = k * t + hi - 1 - npsum + 1
                        if lim > 0:
                            tensor.wait_ge(sem_f, lim)
                    tensor.transpose(
                        pts[g % npsum].ap(),
                        gts[b][:, tt * 128 : (tt + 1) * 128],
                        ident[:],
                    ).then_inc(sem_t, 1)

        @block.vector
        def _(vector):
            for k in range(total):
                b = k % NBUF
                vector.wait_ge(sem_z[b], 16 * nuses(k))
                vector.wait_ge(sem_is[b], 32 * nuses(k))
                if k >= NBUF:
                    vector.wait_ge(sem_o[b], 16 * nuses(k - NBUF))  # zt rewrite vs out read
                for tt in range(t):
                    g = k * t + tt
                    if tt % wgrp == 0:
                        hi = min(tt + wgrp, t)
                        vector.wait_ge(sem_t, k * t + hi)  # transposes done
                    vector.scalar_tensor_tensor(
                        out=zts[b][:, tt * Z : (tt + 1) * Z],
                        in0=pts[g % npsum].ap(),
                        scalar=scts[b][:, tt : tt + 1],
                        in1=zts[b][:, tt * Z : (tt + 1) * Z],
                        op0=mybir.AluOpType.mult,
                        op1=mybir.AluOpType.add,
                    ).then_inc(sem_f, 1)

    # Raw Bass skips Bacc's extended-inst lowering; without it the NEFF
    # compiler sees empty .instr on InstISA subclasses -> "ISA wrong length".
    mybir.codegen_inst_isa_subclasses(nc)
    return nc


def prep_core_inputs(z, idx16, scale, bc, chunk):
    """Host-side layout prep for one core's batch slice."""
    t = chunk // 128
    nch = bc // chunk
    # gather position i -> batch row (i%128)*t + i//128 within the chunk
    i = np.arange(chunk)
    perm = (i % 128) * t + i // 128
    pres = idx16.reshape(nch, chunk)[:, perm]  # [nch, chunk]
    # gather reads index i from (partition i%16, column i//16); replicate x8
    idxw = np.tile(pres.reshape(nch, chunk // 16, 16).transpose(0, 2, 1), (1, 8, 1))
    scw = scale.reshape(nch, 128, t)
    return {
        "z": np.ascontiguousarray(z),
        "idxw": np.ascontiguousarray(idxw),
        "scw": np.ascontiguousarray(scw),
    }


def prep_all_cores(z, a, labels_idx, labels_scale, _chunk=8192):
    a_c = np.ascontiguousarray(np.asarray(a, dtype=np.float32))
    ident = np.eye(128, dtype=np.float32)
    idx16 = np.asarray(labels_idx).astype(np.int16)
    z = np.asarray(z)
    labels_scale = np.asarray(labels_scale)
    ins = []
    for m in range(NCORES):
        s = slice(m * BC, (m + 1) * BC)
        d = prep_core_inputs(z[s], idx16[s], labels_scale[s], BC, _chunk)
        d["a"] = a_c
        d["identw"] = ident
        ins.append(d)
    return ins


def kernel(z, a, labels_idx, labels_scale, _chunk=8192, _trace=False):
    nc = build_program(BC, _chunk)
    ins = prep_all_cores(z, a, labels_idx, labels_scale, _chunk)
    res = run_bass_kernel_spmd(nc, ins, core_ids=list(range(NCORES)), trace=_trace)
    full = np.concatenate([res.results[m]["out"] for m in range(NCORES)], axis=0)
    if _trace:
        return full, res
    return full


# revision 8
# speedup vs baseline: 21.9336x; 21.9336x over previous
"""Embedding-lookup kernel for 8 TRN2 NeuronCores.

Computes out[b, :] = z[b, :] + a[:, idx[b]] * scale[b] for B=1M rows.

Strategy (data-parallel over batch):
  - Each of the 8 cores handles BC = B/8 = 131072 rows.
  - The (128, 512) table `a` lives in SBUF (2KB/partition); the gather
    happens ON-CHIP: GPSIMD's ap_gather reads G[d, i] = a[d, idx[i]]
    (output free dim = batch, partitions = z_dim), TensorE transposes
    each 128x128 tile into PSUM ([batch, z_dim] layout), and DVE fuses
    (g * scale + z) via scalar_tensor_tensor straight out of PSUM.
  - z / out stream contiguously (32KB per-partition runs); the only
    DMA traffic is z in + out back (+2.5MB idx/scale per core), so the
    kernel is limited by the ~128MB/core stream instead of the 131072
    random 512-byte descriptor-bound table reads the previous
    dma_gather version paid (~1.2ms -> this version ~0.5ms).
  - Indices are pre-permuted on the host so ap_gather position
    i = tt*128 + p holds batch row p*t + tt; after the tile transpose,
    PSUM partition p, tile tt lines up with z's SBUF layout.

Raw Bass (no Tile framework), manually triple-buffered: SP issues HWDGE
loads/stores, GPSIMD the gathers, PE the transposes, DVE the fused FMA.
Semaphores count monotonically; NRT's sema_reset preamble re-zeroes
them before every execution.
"""

import contextlib

import numpy as np

import concourse.bass as bass
import concourse.mybir as mybir
from concourse import library_config
from concourse.bass_utils import run_bass_kernel_spmd

F32 = mybir.dt.float32
I16 = mybir.dt.int16

B = 1048576
Z = 128
K = 512
NCORES = 8
BC = B // NCORES  # rows per core
NBUF = 3


def build_program(bc=BC, chunk=8192, repeats=1, gather_n=8192, _ablate=(),
                  nbuf=None, npsum=8, wgrp=4, bench_io=False):
    """Build the single-core Bass program (same module runs SPMD on all cores).

    repeats > 1 re-runs the whole computation (statically unrolled) for
    benchmarking: wall-time slope over repeats isolates on-device time.
    gather_n: indices per ap_gather instruction.
    wgrp: transpose/fma tiles per semaphore wait (amortizes seq waits).
    bench_io: timing-only variant - z/out become Internal DRAM scratch
    (same instruction stream, garbage data) so per-execution transfers
    shrink from ~1.5GB to ~20MB; a tiny `done` external output provides
    completion. Numerically meaningless, structurally identical.
    """
    t = chunk // 128  # column blocks per chunk
    nch = bc // chunk  # chunks per core
    assert bc % chunk == 0 and chunk % 128 == 0
    NBUF = nbuf or globals()["NBUF"]
    gather_n = min(gather_n, chunk)
    nsub = chunk // gather_n
    assert chunk % gather_n == 0 and gather_n % 128 == 0
    total = nch * repeats
    ntiles = total * t  # global 128-row tile count

    nc = bass.Bass()
    a_p = nc.declare_dram_parameter("a", [Z, K], F32, isOutput=False)
    ident_p = nc.declare_dram_parameter("identw", [128, 128], F32, isOutput=False)
    if bench_io:
        z = nc.dram_tensor("z", [bc, Z], F32, kind="Internal")
        idxw = nc.declare_dram_parameter("idxw", [nch, 128, chunk // 16], I16, isOutput=False)
        scw = nc.declare_dram_parameter("scw", [nch, 128, t], F32, isOutput=False)
        out = nc.dram_tensor("out", [bc, Z], F32, kind="Internal")
        done = nc.declare_dram_parameter("done", [1, 64], I16, isOutput=True)
    else:
        z = nc.declare_dram_parameter("z", [bc, Z], F32, isOutput=False)
        idxw = nc.declare_dram_parameter("idxw", [nch, 128, chunk // 16], I16, isOutput=False)
        scw = nc.declare_dram_parameter("scw", [nch, 128, t], F32, isOutput=False)
        out = nc.declare_dram_parameter("out", [bc, Z], F32, isOutput=True)
        done = None

    # chunk-row b = p*t + tt lives at SBUF (partition p, column block tt)
    z_v = z.ap().rearrange("(c p tt) d -> c p (tt d)", p=128, tt=t)
    o_v = out.ap().rearrange("(c p tt) d -> c p (tt d)", p=128, tt=t)

    with contextlib.ExitStack() as ctx:
        zts = [
            ctx.enter_context(nc.sbuf_tensor(f"zt{i}", [128, t * Z], F32))
            for i in range(NBUF)
        ]
        gts = [
            ctx.enter_context(nc.sbuf_tensor(f"gt{i}", [128, chunk], F32))
            for i in range(NBUF)
        ]
        idxts = [
            ctx.enter_context(nc.sbuf_tensor(f"idxt{i}", [128, chunk // 16], I16))
            for i in range(NBUF)
        ]
        scts = [
            ctx.enter_context(nc.sbuf_tensor(f"sct{i}", [128, t], F32))
            for i in range(NBUF)
        ]
        tab = ctx.enter_context(nc.sbuf_tensor("tab", [128, K], F32))
        ident = ctx.enter_context(nc.sbuf_tensor("ident", [128, 128], F32))
        pts = [
            ctx.enter_context(nc.psum_tensor(f"pt{i}", [128, 128], F32))
            for i in range(npsum)
        ]
        sem_is = [ctx.enter_context(nc.semaphore(f"sem_is{i}")) for i in range(NBUF)]
        sem_z = [ctx.enter_context(nc.semaphore(f"sem_z{i}")) for i in range(NBUF)]
        sem_g = [ctx.enter_context(nc.semaphore(f"sem_g{i}")) for i in range(NBUF)]
        sem_o = [ctx.enter_context(nc.semaphore(f"sem_o{i}")) for i in range(NBUF)]
        sem_tab = ctx.enter_context(nc.semaphore("sem_tab"))
        sem_id = ctx.enter_context(nc.semaphore("sem_id"))
        sem_t = ctx.enter_context(nc.semaphore("sem_t"))  # PE transposes, +1/tile
        sem_f = ctx.enter_context(nc.semaphore("sem_f"))  # DVE fmas, +1/tile
        block = ctx.enter_context(nc.Block())

        def nuses(j):  # completed uses of slot j%NBUF's sems after chunk j
            return j // NBUF + 1

        # ablation helpers: when a stage is removed its semaphore never
        # increments, so consumers must also skip waiting on it
        has_g = "nogather" not in _ablate
        has_t = "notrans" not in _ablate
        has_f = "nofma" not in _ablate
        has_zl = "noz" not in _ablate
        has_os = "noout" not in _ablate

        @block.sync
        def _(sync):
            sync.dma_start(out=tab[:], in_=a_p.ap()).then_inc(sem_tab, 16)
            sync.dma_start(out=ident[:], in_=ident_p.ap()).then_inc(sem_id, 16)
            for k in range(total):
                c = k % nch
                b = k % NBUF
                if k >= NBUF:
                    # slot reuse: gather(k-NBUF), fma(k-NBUF), out(k-NBUF) done
                    if has_g:
                        sync.wait_ge(sem_g[b], nsub * nuses(k - NBUF))
                    if has_f:
                        sync.wait_ge(sem_f, (k - NBUF + 1) * t)
                    if has_os:
                        sync.wait_ge(sem_o[b], 16 * nuses(k - NBUF))
                sync.dma_start(out=idxts[b][:], in_=idxw.ap()[c]).then_inc(sem_is[b], 16)
                sync.dma_start(out=scts[b][:], in_=scw.ap()[c]).then_inc(sem_is[b], 16)
                if has_zl:
                    sync.dma_start(out=zts[b][:], in_=z_v[c]).then_inc(sem_z[b], 16)
                if k >= 2 and has_os:
                    j = k - 2  # store lags loads by 2 chunks
                    if has_f:
                        sync.wait_ge(sem_f, (j + 1) * t)
                    sync.dma_start(out=o_v[j % nch], in_=zts[j % NBUF][:]).then_inc(
                        sem_o[j % NBUF], 16
                    )
            if has_os:
                for j in range(max(total - 2, 0), total):
                    if has_f:
                        sync.wait_ge(sem_f, (j + 1) * t)
                    sync.dma_start(out=o_v[j % nch], in_=zts[j % NBUF][:]).then_inc(
                        sem_o[j % NBUF], 16
                    )
                for b in range(NBUF):
                    count_b = len([j for j in range(total) if j % NBUF == b])
                    if count_b:
                        sync.wait_ge(sem_o[b], 16 * count_b)
            if done is not None:
                if has_g and not has_os:
                    for b in range(NBUF):
                        cnt = len([j for j in range(total) if j % NBUF == b])
                        if cnt:
                            sync.wait_ge(sem_g[b], nsub * cnt)
                if has_f and not has_os:
                    sync.wait_ge(sem_f, total * t)
                sync.dma_start(out=done.ap(), in_=idxts[0][:1, :64]).then_inc(
                    sem_is[0], 16
                )
                sync.wait_ge(sem_is[0], 32 * nuses(total - 1) + 16)

        @block.gpsimd
        def _(gpsimd):
            if has_g:
                gpsimd.load_library(library_config.ap_gather)
                gpsimd.wait_ge(sem_tab, 16)
                for k in range(total):
                    b = k % NBUF
                    gpsimd.wait_ge(sem_is[b], 32 * nuses(k))
                    if k >= NBUF and has_t:
                        # gt slot reuse: PE finished transposing chunk k-NBUF
                        gpsimd.wait_ge(sem_t, (k - NBUF + 1) * t)
                    for s in range(nsub):
                        gpsimd.ap_gather(
                            out_ap=gts[b][:, s * gather_n : (s + 1) * gather_n],
                            in_ap=tab[:],
                            idxs_ap=idxts[b][:, s * (gather_n // 16) : (s + 1) * (gather_n // 16)],
                            channels=128,
                            num_elems=K,
                            d=1,
                            num_idxs=gather_n,
                        ).then_inc(sem_g[b], 1)

        @block.tensor
        def _(tensor):
            if has_t:
                tensor.wait_ge(sem_id, 16)
                for k in range(total):
                    b = k % NBUF
                    spt = gather_n // 128  # tiles covered per sub-gather
                    for tt in range(t):
                        g = k * t + tt  # global tile index
                        if tt % wgrp == 0:
                            hi = min(tt + wgrp, t)
                            # gathers covering tiles tt..hi-1 done
                            if has_g:
                                need = (hi + spt - 1) // spt
                                tensor.wait_ge(sem_g[b], nsub * (nuses(k) - 1) + need)
                            # PSUM tile reuse: fma consumed tile g+wgrp-1-npsum
                            lim = k * t + hi - 1 - npsum + 1
                            if lim > 0 and has_f:
                                tensor.wait_ge(sem_f, lim)
                        tensor.transpose(
                            pts[g % npsum].ap(),
                            gts[b][:, tt * 128 : (tt + 1) * 128],
                            ident[:],
                        ).then_inc(sem_t, 1)

        @block.vector
        def _(vector):
            for k in range(total if has_f else 0):
                b = k % NBUF
                if has_zl:
                    vector.wait_ge(sem_z[b], 16 * nuses(k))
                vector.wait_ge(sem_is[b], 32 * nuses(k))
                if k >= NBUF and has_os:
                    vector.wait_ge(sem_o[b], 16 * nuses(k - NBUF))  # zt rewrite vs out read
                for tt in range(t):
                    g = k * t + tt
                    if tt % wgrp == 0 and has_t:
                        hi = min(tt + wgrp, t)
                        vector.wait_ge(sem_t, k * t + hi)  # transposes done
                    vector.scalar_tensor_tensor(
                        out=zts[b][:, tt * Z : (tt + 1) * Z],
                        in0=pts[g % npsum].ap(),
                        scalar=scts[b][:, tt : tt + 1],
                        in1=zts[b][:, tt * Z : (tt + 1) * Z],
                        op0=mybir.AluOpType.mult,
                        op1=mybir.AluOpType.add,
                    ).then_inc(sem_f, 1)

    # Raw Bass skips Bacc's extended-inst lowering; without it the NEFF
    # compiler sees empty .instr on InstISA subclasses -> "ISA wrong length".
    mybir.codegen_inst_isa_subclasses(nc)
    return nc


def prep_core_inputs(z, idx16, scale, bc, chunk):
    """Host-side layout prep for one core's batch slice."""
    t = chunk // 128
    nch = bc // chunk
    # gather position i -> batch row (i%128)*t + i//128 within the chunk
    i = np.arange(chunk)
    perm = (i % 128) * t + i // 128
    pres = idx16.reshape(nch, chunk)[:, perm]  # [nch, chunk]
    # gather reads index i from (partition i%16, column i//16); replicate x8
    idxw = np.tile(pres.reshape(nch, chunk // 16, 16).transpose(0, 2, 1), (1, 8, 1))
    scw = scale.reshape(nch, 128, t)
    return {
        "z": np.ascontiguousarray(z),
        "idxw": np.ascontiguousarray(idxw),
        "scw": np.ascontiguousarray(scw),
    }


def prep_all_cores(z, a, labels_idx, labels_scale, _chunk=8192):
    a_c = np.ascontiguousarray(np.asarray(a, dtype=np.float32))
    ident = np.eye(128, dtype=np.float32)
    idx16 = np.asarray(labels_idx).astype(np.int16)
    z = np.asarray(z)
    labels_scale = np.asarray(labels_scale)
    ins = []
    for m in range(NCORES):
        s = slice(m * BC, (m + 1) * BC)
        d = prep_core_inputs(z[s], idx16[s], labels_scale[s], BC, _chunk)
        d["a"] = a_c
        d["identw"] = ident
        ins.append(d)
    return ins


def kernel(z, a, labels_idx, labels_scale, _chunk=8192, _trace=False):
    nc = build_program(BC, _chunk)
    ins = prep_all_cores(z, a, labels_idx, labels_scale, _chunk)
    res = run_bass_kernel_spmd(nc, ins, core_ids=list(range(NCORES)), trace=_trace)
    full = np.concatenate([res.results[m]["out"] for m in range(NCORES)], axis=0)
    if _trace:
        return full, res
    return full
